# revision 1
# baseline (speedup 1.0000x reference)
"""DeepseekV3 MoE layer on 8 Trainium2 NeuronCores (Bass/Tile).

Sharding:
  - Router: data-parallel (each core routes its own T/8=512 tokens, fp32,
    selection done on exact logits), then AllGather of per-token
    (sel-mask, weight) -> every core knows the full routing.
  - Capacity ranks: per-expert running count over tokens via DVE prefix scan;
    rank <= C survives (matches the reference's stable-sort capacity drop,
    because top-k experts within a token are distinct -> per-expert arrival
    order is token order, and slot order within an expert doesn't affect the
    output).
  - Routed experts: expert-parallel, 4 experts/core.  Token rows are
    dma_gather'ed by compacted slot lists (capacity C=160, padded to 256 per
    expert), GEMM'd, weighted, and dma_scatter_add'ed into a [T, D] partial.
  - Combine: ReduceScatter(add) of partials -> each core owns its 512-token
    slice; adds its locally computed shared-expert MLP and writes the output
    slice.

kernel(**inputs) takes the full unsharded inputs and returns the full
[B, S, D] output.  Self-contained: hardcodes all shapes.
"""

import os
import sys

for _p in ("/opt/trn_rl_repo", "/opt/pypackages"):
    if _p not in sys.path:
        sys.path.insert(0, _p)

import numpy as np

# ---------------------------------------------------------------- constants
B, S, D = 2, 2048, 2048
T = B * S                  # 4096 tokens
I = 1024                   # routed expert intermediate
E = 32                     # routed experts
K = 4                      # experts per token
NG = 8                     # groups
GS = E // NG               # experts per group = 4
TKG = 3                    # top-k groups
ISH = 2048                 # shared expert intermediate (I * n_shared)
SCALE = 2.5
C = 160                    # capacity = ceil(1.25 * T / E)
CP = 256                   # per-expert slot padding (128-aligned)
NCORES = 8
EL = E // NCORES           # local experts per core = 4
TL = T // NCORES           # local tokens per core = 512
NSLOT = EL * CP            # padded slots per core = 1024

# "f32" | "f32r" | "bf16" : dtype/mode of the heavy GEMMs (router stays f32)
GEMM_MODE = os.environ.get("BASS_MOE_GEMM_MODE", "f32")


# ---------------------------------------------------------------- builder
def _build(gemm_mode: str):
    import concourse.bass as bass
    import concourse.bacc as bacc
    import concourse.mybir as mybir
    import concourse.tile as tile
    from concourse import masks
    from contextlib import ExitStack

    dt = mybir.dt
    Alu = mybir.AluOpType
    Act = mybir.ActivationFunctionType

    f32 = dt.float32
    bf16 = dt.bfloat16
    wdt = bf16 if gemm_mode == "bf16" else f32

    def mm_cast(ap):
        if gemm_mode == "f32r":
            return ap.bitcast(dt.float32r)
        return ap

    nc = bacc.Bacc(None, num_devices=NCORES, num_swdge_queues=1)
    groups = [list(range(NCORES))]

    # ---------------- I/O ----------------
    x_full = nc.dram_tensor("x_full", [T, D], wdt, kind="ExternalInput")
    x_own = nc.dram_tensor("x_own", [TL, D], f32, kind="ExternalInput")
    rwT = nc.dram_tensor("rwT", [D, E], f32, kind="ExternalInput")
    ebias = nc.dram_tensor("ebias", [1, E], f32, kind="ExternalInput")
    sloc = nc.dram_tensor("sloc", [2 * E, 36], f32, kind="ExternalInput")
    wg = nc.dram_tensor("wg", [EL, D, I], wdt, kind="ExternalInput")
    wu = nc.dram_tensor("wu", [EL, D, I], wdt, kind="ExternalInput")
    wd = nc.dram_tensor("wd", [EL, I, D], wdt, kind="ExternalInput")
    sgT = nc.dram_tensor("sgT", [D, ISH], wdt, kind="ExternalInput")
    suT = nc.dram_tensor("suT", [D, ISH], wdt, kind="ExternalInput")
    sdT = nc.dram_tensor("sdT", [ISH, D], wdt, kind="ExternalInput")
    out = nc.dram_tensor("out", [TL, D], f32, kind="ExternalOutput")

    # ---------------- internal DRAM ----------------
    selw_own = nc.dram_tensor("selw_own", [TL, 2 * E], f32)
    selw_all = nc.dram_tensor("selw_all", [T, 2 * E], f32, addr_space="Shared")
    partial = nc.dram_tensor("partial", [T, D], f32)
    rs_out = nc.dram_tensor("rs_out", [TL, D], f32)
    shr_out = nc.dram_tensor("shr_out", [TL, D], f32)
    idx_dram = nc.dram_tensor("idx_dram", [16, EL * 16], dt.int16)
    at_dram = nc.dram_tensor("at_dram", [EL, T], f32)
    nf_dram = nc.dram_tensor("nf_dram", [1, EL], f32)
    aw_dram = nc.dram_tensor("aw_dram", [EL, T], f32)

    DC = D // 128            # 16 d-chunks
    IC = I // 128            # 8  i-chunks
    MC = ISH // 128          # 16 shared-intermediate chunks
    TT = TL // 128           # 4 own-token tiles
    NT = T // 128            # 32 all-token tiles
    CH = 4                   # routing-table token chunks
    CT = T // CH             # 1024 tokens per chunk

    with tile.TileContext(nc) as tc, ExitStack() as ctx:
        consts = ctx.enter_context(tc.tile_pool(name="consts", bufs=1))
        work = ctx.enter_context(tc.tile_pool(name="work", bufs=2))
        psum_t = ctx.enter_context(
            tc.tile_pool(name="psum_t", bufs=2, space="PSUM"))
        psum_g = ctx.enter_context(
            tc.tile_pool(name="psum_g", bufs=2, space="PSUM"))
        psum_u = ctx.enter_context(
            tc.tile_pool(name="psum_u", bufs=2, space="PSUM"))
        psum_y = ctx.enter_context(
            tc.tile_pool(name="psum_y", bufs=2, space="PSUM"))
        persist = ctx.enter_context(tc.tile_pool(name="persist", bufs=1))
        wstream = ctx.enter_context(tc.tile_pool(name="wstream", bufs=2))

        # ---------------- constants ----------------
        ident = consts.tile([128, 128], f32)
        masks.make_identity(nc, ident[:])
        if wdt != f32:
            ident_w = consts.tile([128, 128], wdt)
            nc.vector.tensor_copy(ident_w[:], ident[:])
        else:
            ident_w = ident

        ebias_b = consts.tile([128, E], f32)
        nc.sync.dma_start(ebias_b[:], ebias[0:1, :].broadcast_to([128, E]))

        negbuf = consts.tile([128, E], f32)
        nc.gpsimd.memset(negbuf[:], -1e30)

        iota16_i = consts.tile([16, 16], dt.int32)
        nc.gpsimd.iota(iota16_i[:], pattern=[[16, 16]], base=0,
                       channel_multiplier=1)
        iota16 = consts.tile([16, 16], f32)
        nc.vector.tensor_copy(iota16[:], iota16_i[:])

        # zero-fill the [T, D] partial early (overlaps with compute)
        zt = consts.tile([128, 512], f32)
        nc.gpsimd.memset(zt[:], 0.0)
        for r in range(NT):
            for zc in range(D // 512):
                nc.sync.dma_start(
                    partial[r * 128:(r + 1) * 128,
                            zc * 512:(zc + 1) * 512], zt[:])

        # ---------------- P1: transpose own tokens -> xT [128, DC, TL] ----
        xtp_cm = tc.tile_pool(name="xtp", bufs=1)
        xtp = xtp_cm.__enter__()
        xT = xtp.tile([128, DC, TL], f32)
        for tt in range(TT):
            for dc2 in range(DC // 2):
                xtile = work.tile([128, 256], f32, tag="xtile")
                nc.sync.dma_start(
                    xtile[:],
                    x_own[tt * 128:(tt + 1) * 128, dc2 * 256:(dc2 + 1) * 256])
                for h in range(2):
                    dc = dc2 * 2 + h
                    pt = psum_t.tile([128, 128], f32, tag="pt")
                    nc.tensor.transpose(
                        pt[:], xtile[:, h * 128:(h + 1) * 128], ident[:])
                    nc.vector.tensor_copy(
                        xT[:, dc, tt * 128:(tt + 1) * 128], pt[:])
        if wdt != f32:
            xTw = xtp.tile([128, DC, TL], wdt)
            for dc in range(DC):
                nc.vector.tensor_copy(xTw[:, dc, :], xT[:, dc, :])
        else:
            xTw = xT

        # ---------------- P2: router on own tokens (fp32/exact) -----------
        rwT_sb = consts.tile([128, DC, E], f32)
        nc.sync.dma_start(
            rwT_sb[:], rwT[:].rearrange("(c p) e -> p c e", p=128))

        for tt in range(TT):
            ps = psum_t.tile([128, E], f32, tag="pt")
            for dc in range(DC):
                nc.tensor.matmul(
                    ps[:], xT[:, dc, tt * 128:(tt + 1) * 128], rwT_sb[:, dc, :],
                    start=(dc == 0), stop=(dc == DC - 1))
            L = work.tile([128, E], f32, tag="rL")
            nc.vector.tensor_copy(L[:], ps[:])
            Ssig = work.tile([128, E], f32, tag="rS")
            nc.scalar.activation(Ssig[:], ps[:], Act.Sigmoid)
            Sb = work.tile([128, E], f32, tag="rSb")
            nc.vector.tensor_tensor(Sb[:], Ssig[:], ebias_b[:], op=Alu.add)

            # group score = top-2 sum per group = max over pair sums
            Sv = Sb[:].rearrange("p (g i) -> p g i", i=GS)
            gs = work.tile([128, NG], f32, tag="rGS")
            tmp = work.tile([128, NG], f32, tag="rtmp")
            nc.vector.tensor_tensor(gs[:], Sv[:, :, 0], Sv[:, :, 1], op=Alu.add)
            for (a, b) in [(0, 2), (0, 3), (1, 2), (1, 3), (2, 3)]:
                nc.vector.tensor_tensor(
                    tmp[:], Sv[:, :, a], Sv[:, :, b], op=Alu.add)
                nc.vector.tensor_tensor(gs[:], gs[:], tmp[:], op=Alu.max)

            m8g = work.tile([128, 8], f32, tag="rm8g")
            nc.vector.max(m8g[:], gs[:])
            gmask = work.tile([128, NG], f32, tag="rgm")
            nc.vector.tensor_scalar(
                gmask[:], gs[:], m8g[:, TKG - 1:TKG], None, op0=Alu.is_ge)

            emask = work.tile([128, E], f32, tag="rem")
            emv = emask[:].rearrange("p (g i) -> p g i", i=GS)
            for r in range(GS):
                nc.vector.tensor_copy(emv[:, :, r], gmask[:])

            # top-4 experts among unmasked, compared on exact logits
            emask8 = work.tile([128, E], dt.uint8, tag="rem8")
            nc.vector.tensor_copy(emask8[:], emask[:])
            ml = work.tile([128, E], f32, tag="rml")
            nc.vector.tensor_copy(ml[:], negbuf[:])
            nc.vector.copy_predicated(ml[:], emask8[:], L[:])
            m8e = work.tile([128, 8], f32, tag="rm8e")
            nc.vector.max(m8e[:], ml[:])
            sel = work.tile([128, E], f32, tag="rsel")
            nc.vector.tensor_scalar(
                sel[:], ml[:], m8e[:, K - 1:K], None, op0=Alu.is_ge)

            wm = work.tile([128, E], f32, tag="rwm")
            nc.vector.tensor_tensor(wm[:], Ssig[:], sel[:], op=Alu.mult)
            den = work.tile([128, 1], f32, tag="rden")
            nc.vector.tensor_reduce(
                den[:], wm[:], axis=mybir.AxisListType.X, op=Alu.add)
            nc.vector.tensor_scalar(den[:], den[:], 1e-20, None, op0=Alu.add)
            winv = work.tile([128, 1], f32, tag="rwinv")
            nc.vector.reciprocal(winv[:], den[:])

            sw = work.tile([128, 2 * E], f32, tag="rsw")
            nc.vector.tensor_copy(sw[:, 0:E], sel[:])
            nc.vector.tensor_scalar(
                sw[:, E:2 * E], wm[:], winv[:, 0:1], SCALE,
                op0=Alu.mult, op1=Alu.mult)
            nc.sync.dma_start(selw_own[tt * 128:(tt + 1) * 128, :], sw[:])

        # ---------------- P8a: shared expert gate/up (independent) --------
        HsT = persist.tile([128, MC, TL], wdt)
        for mc in range(MC):
            sg_t = wstream.tile([128, DC, 128], wdt, tag="wst")
            nc.sync.dma_start(
                sg_t[:],
                sgT[:].rearrange("(c p) i -> p c i", p=128)
                [:, :, mc * 128:(mc + 1) * 128])
            su_t = wstream.tile([128, DC, 128], wdt, tag="wst2")
            nc.sync.dma_start(
                su_t[:],
                suT[:].rearrange("(c p) i -> p c i", p=128)
                [:, :, mc * 128:(mc + 1) * 128])
            pg = psum_g.tile([128, TL], f32, tag="pg")
            pu = psum_u.tile([128, TL], f32, tag="pu")
            for dc in range(DC):
                nc.tensor.matmul(
                    pg[:], mm_cast(sg_t[:, dc, :]), mm_cast(xTw[:, dc, :]),
                    start=(dc == 0), stop=(dc == DC - 1))
            for dc in range(DC):
                nc.tensor.matmul(
                    pu[:], mm_cast(su_t[:, dc, :]), mm_cast(xTw[:, dc, :]),
                    start=(dc == 0), stop=(dc == DC - 1))
            sig = work.tile([128, TL], f32, tag="ssig")
            nc.scalar.activation(sig[:], pg[:], Act.Sigmoid)
            sil = work.tile([128, TL], wdt, tag="ssil")
            nc.vector.tensor_tensor(sil[:], sig[:], pg[:], op=Alu.mult)
            nc.vector.tensor_tensor(HsT[:, mc, :], sil[:], pu[:], op=Alu.mult)

        xtp_cm.__exit__(None, None, None)

        # ---------------- P3: AllGather routing ----------------
        nc.gpsimd.collective_compute(
            "AllGather", Alu.bypass, replica_groups=groups,
            ins=[selw_own[:]], outs=[selw_all[:]])

        # ---------------- P4: routing tables (chunked over tokens) --------
        sloc_sb = consts.tile([64, 36], f32)
        nc.sync.dma_start(sloc_sb[:], sloc[:])

        sgin_t = persist.tile([16, EL, T // 16], f32)
        sgin_w = persist.tile([16, EL, T // 16], f32)
        carry = persist.tile([EL, 1], f32)
        nc.gpsimd.memset(carry[:], 0.0)

        route_cm = tc.tile_pool(name="route", bufs=1)
        route = route_cm.__enter__()
        for q in range(CH):
            selwT_c = route.tile([64, CT // 128, 128], f32, tag="selwT")
            for j in range(CT // 128):
                tt = q * (CT // 128) + j
                swt = work.tile([128, 2 * E], f32, tag="swt")
                nc.sync.dma_start(
                    swt[:], selw_all[tt * 128:(tt + 1) * 128, :])
                pt = psum_t.tile([64, 128], f32, tag="pt")
                nc.tensor.transpose(pt[:], swt[:], ident[:])
                nc.vector.tensor_copy(selwT_c[:, j, :], pt[:])

            SW_sel = route.tile([EL, CT], f32, tag="SWsel")
            SW_w = route.tile([EL, CT], f32, tag="SWw")
            for h in range(CT // 512):
                pswl = psum_g.tile([36, 512], f32, tag="pg")
                nc.tensor.matmul(
                    pswl[:], sloc_sb[:], selwT_c[:, 4 * h:4 * (h + 1), :],
                    start=True, stop=True)
                nc.vector.tensor_copy(
                    SW_sel[:, h * 512:(h + 1) * 512], pswl[0:EL, :])
                nc.vector.tensor_copy(
                    SW_w[:, h * 512:(h + 1) * 512], pswl[32:36, :])

            rank_c = route.tile([EL, CT], f32, tag="rankc")
            nc.vector.tensor_tensor_scan(
                rank_c[:], SW_sel[:], SW_sel[:], carry[:, 0:1],
                op0=Alu.add, op1=Alu.bypass)
            nc.vector.tensor_copy(carry[:], rank_c[:, CT - 1:CT])

            fsel_c = route.tile([EL, CT], f32, tag="fselc")
            nc.vector.tensor_scalar(
                fsel_c[:], rank_c[:], float(C), None, op0=Alu.is_le)
            nc.vector.tensor_tensor(
                fsel_c[:], fsel_c[:], SW_sel[:], op=Alu.mult)

            iota_i = route.tile([EL, CT], dt.int32, tag="iotai")
            nc.gpsimd.iota(iota_i[:], pattern=[[1, CT]], base=1 + q * CT,
                           channel_multiplier=0)
            iota_f = route.tile([EL, CT], f32, tag="iotaf")
            nc.vector.tensor_copy(iota_f[:], iota_i[:])

            At_c = route.tile([EL, CT], f32, tag="Atc")
            nc.vector.tensor_tensor(At_c[:], fsel_c[:], iota_f[:], op=Alu.mult)
            nc.vector.tensor_scalar(At_c[:], At_c[:], 1.0, None,
                                    op0=Alu.subtract)

            fsel8 = route.tile([EL, CT], dt.uint8, tag="fsel8")
            nc.vector.tensor_copy(fsel8[:], fsel_c[:])
            Aw_c = route.tile([EL, CT], f32, tag="Awc")
            nc.gpsimd.memset(Aw_c[:], -1.0)
            nc.vector.copy_predicated(Aw_c[:], fsel8[:], SW_w[:])

            nc.sync.dma_start(at_dram[:, q * CT:(q + 1) * CT], At_c[:])
            nc.sync.dma_start(aw_dram[:, q * CT:(q + 1) * CT], Aw_c[:])

        for e in range(EL):
            nc.sync.dma_start(
                sgin_t[:, e, :],
                at_dram[e].rearrange("(c b) -> b c", b=16))
            nc.sync.dma_start(
                sgin_w[:, e, :],
                aw_dram[e].rearrange("(c b) -> b c", b=16))

        # per-expert compaction -> slot lists + weights
        idx16s = persist.tile([16, EL * 16], dt.int16)   # 16-row wrapped
        idx16 = persist.tile([128, EL * 16], dt.int16)   # replicated to 128
        w_col = persist.tile([128, 2 * EL], f32)

        sgtoks, sgws = [], []
        for e in range(EL):
            sgtok = work.tile([16, 16], f32, tag=f"sgtok{e}")
            nft = work.tile([1, 1], dt.uint32, tag=f"nft{e}")
            nc.gpsimd.sparse_gather(sgtok[:], sgin_t[:, e, :], num_found=nft[:])
            sgw = work.tile([16, 16], f32, tag=f"sgw{e}")
            nfw = work.tile([1, 1], dt.uint32, tag=f"nfw{e}")
            nc.gpsimd.sparse_gather(sgw[:], sgin_w[:, e, :], num_found=nfw[:])
            nf_f = work.tile([1, 1], f32, tag=f"nf_f{e}")
            nc.vector.tensor_copy(nf_f[:], nft[:])
            nc.sync.dma_start(nf_dram[0:1, e:e + 1], nf_f[:])
            sgtoks.append(sgtok)
            sgws.append(sgw)

        for e in range(EL):
            sgtok, sgw = sgtoks[e], sgws[e]
            nf16 = work.tile([16, 1], f32, tag=f"nf16{e}")
            nc.sync.dma_start(
                nf16[:], nf_dram[0:1, e:e + 1].broadcast_to([16, 1]))
            vm = work.tile([16, 16], f32, tag=f"vm{e}")
            nc.vector.tensor_scalar(
                vm[:], iota16[:], nf16[:, 0:1], None, op0=Alu.is_lt)

            tokm = work.tile([16, 16], f32, tag=f"tokm{e}")
            nc.vector.tensor_tensor(tokm[:], sgtok[:], vm[:], op=Alu.mult)
            nc.vector.tensor_copy(idx16s[:, 16 * e:16 * (e + 1)], tokm[:])

            wsl = work.tile([16, 16], f32, tag=f"wsl{e}")
            nc.vector.tensor_tensor(wsl[:], sgw[:], vm[:], op=Alu.mult)
            ptw = psum_t.tile([16, 16], f32, tag="pt")
            nc.tensor.transpose(ptw[:], wsl[:], ident[:16, :16])
            wt16 = work.tile([16, 16], f32, tag=f"wt16{e}")
            nc.vector.tensor_copy(wt16[:], ptw[:])
            nc.sync.dma_start(w_col[:, 2 * e:2 * e + 1], wt16[0:8, :])
            nc.sync.dma_start(w_col[:, 2 * e + 1:2 * e + 2], wt16[8:16, :])

        # replicate the 16-row wrapped index block to all 128 partitions
        nc.sync.dma_start(idx_dram[:], idx16s[:])
        for r in range(8):
            nc.sync.dma_start(idx16[16 * r:16 * (r + 1), :], idx_dram[:])

        route_cm.__exit__(None, None, None)

        # ---------------- P5..P7: dispatch + routed expert GEMMs ----------
        with tc.tile_pool(name="dpXP", bufs=2) as dpXP, \
                tc.tile_pool(name="dpXPT", bufs=1) as dpXPT, \
                tc.tile_pool(name="dpHT", bufs=1) as dpHT, \
                tc.tile_pool(name="dpWD", bufs=1) as dpWD:
            for e in range(EL):
                XP = dpXP.tile([128, CP // 128, D], wdt, tag="XP")
                nc.gpsimd.dma_gather(
                    XP[:], x_full[:], idx16[:, 16 * e:16 * (e + 1)],
                    CP, CP, D, queue_num=0)
                XPT = dpXPT.tile([128, DC, CP], wdt, tag="XPT")
                for col in range(CP // 128):
                    for dc in range(DC):
                        ptx = psum_t.tile([128, 128], wdt, tag="pt")
                        nc.tensor.transpose(
                            ptx[:], XP[:, col, dc * 128:(dc + 1) * 128],
                            ident_w[:])
                        nc.vector.tensor_copy(
                            XPT[:, dc, col * 128:(col + 1) * 128], ptx[:])

                HT = dpHT.tile([128, IC, CP], wdt, tag="HT")
                for ic in range(IC):
                    wg_t = wstream.tile([128, DC, 128], wdt, tag="wst")
                    nc.sync.dma_start(
                        wg_t[:],
                        wg[e].rearrange("(c p) i -> p c i", p=128)
                        [:, :, ic * 128:(ic + 1) * 128])
                    wu_t = wstream.tile([128, DC, 128], wdt, tag="wst2")
                    nc.sync.dma_start(
                        wu_t[:],
                        wu[e].rearrange("(c p) i -> p c i", p=128)
                        [:, :, ic * 128:(ic + 1) * 128])
                    pg = psum_g.tile([128, CP], f32, tag="pg")
                    pu = psum_u.tile([128, CP], f32, tag="pu")
                    for dc in range(DC):
                        nc.tensor.matmul(
                            pg[:], mm_cast(wg_t[:, dc, :]),
                            mm_cast(XPT[:, dc, :]),
                            start=(dc == 0), stop=(dc == DC - 1))
                    for dc in range(DC):
                        nc.tensor.matmul(
                            pu[:], mm_cast(wu_t[:, dc, :]),
                            mm_cast(XPT[:, dc, :]),
                            start=(dc == 0), stop=(dc == DC - 1))
                    sig = work.tile([128, CP], f32, tag="esig")
                    nc.scalar.activation(sig[:], pg[:], Act.Sigmoid)
                    sil = work.tile([128, CP], wdt, tag="esil")
                    nc.vector.tensor_tensor(sil[:], sig[:], pg[:], op=Alu.mult)
                    nc.vector.tensor_tensor(
                        HT[:, ic, :], sil[:], pu[:], op=Alu.mult)

                for dc4 in range(D // 512):
                    wd_t = dpWD.tile([128, IC, 512], wdt, tag="wst3")
                    nc.sync.dma_start(
                        wd_t[:],
                        wd[e].rearrange("(c p) d -> p c d", p=128)
                        [:, :, dc4 * 512:(dc4 + 1) * 512])
                    for sb in range(2):
                        py = psum_y.tile([128, 512], f32, tag="py")
                        for ic in range(IC):
                            nc.tensor.matmul(
                                py[:],
                                mm_cast(HT[:, ic, sb * 128:(sb + 1) * 128]),
                                mm_cast(wd_t[:, ic, :]),
                                start=(ic == 0), stop=(ic == IC - 1))
                        yw = work.tile([128, 512], f32, tag="yw")
                        nc.vector.tensor_scalar(
                            yw[:], py[:],
                            w_col[:, 2 * e + sb:2 * e + sb + 1], None,
                            op0=Alu.mult)
                        nc.gpsimd.dma_scatter_add(
                            partial[:, dc4 * 512:(dc4 + 1) * 512],
                            yw[:].rearrange("p (a f) -> p a f", a=1),
                            idx16[:, 16 * e + 8 * sb:16 * e + 8 * sb + 8],
                            128, 128, 512, elem_step=D, queue_num=0)

        # ---------------- P8b: shared expert down proj -> shr_out ---------
        with tc.tile_pool(name="sdpool", bufs=1) as sdpool:
            for dc4 in range(D // 512):
                sd_t = sdpool.tile([128, MC, 512], wdt, tag="wsd")
                nc.sync.dma_start(
                    sd_t[:],
                    sdT[:].rearrange("(c p) d -> p c d", p=128)
                    [:, :, dc4 * 512:(dc4 + 1) * 512])
                for tb in range(TT):
                    po = psum_y.tile([128, 512], f32, tag="py")
                    for mc in range(MC):
                        nc.tensor.matmul(
                            po[:], mm_cast(HsT[:, mc, tb * 128:(tb + 1) * 128]),
                            mm_cast(sd_t[:, mc, :]),
                            start=(mc == 0), stop=(mc == MC - 1))
                    ot = work.tile([128, 512], f32, tag="ot")
                    nc.vector.tensor_copy(ot[:], po[:])
                    nc.sync.dma_start(
                        shr_out[tb * 128:(tb + 1) * 128,
                                dc4 * 512:(dc4 + 1) * 512], ot[:])

        # ---------------- P9: ReduceScatter + final add ----------------
        nc.gpsimd.collective_compute(
            "ReduceScatter", Alu.add, replica_groups=groups,
            ins=[partial[:]], outs=[rs_out[:]])

        for tb in range(TT):
            for dc4 in range(D // 512):
                rst = work.tile([128, 512], f32, tag="rst")
                nc.sync.dma_start(
                    rst[:], rs_out[tb * 128:(tb + 1) * 128,
                                   dc4 * 512:(dc4 + 1) * 512])
                sht = work.tile([128, 512], f32, tag="sht")
                nc.sync.dma_start(
                    sht[:], shr_out[tb * 128:(tb + 1) * 128,
                                    dc4 * 512:(dc4 + 1) * 512])
                fin = work.tile([128, 512], f32, tag="fin")
                nc.vector.tensor_tensor(fin[:], sht[:], rst[:], op=Alu.add)
                nc.sync.dma_start(
                    out[tb * 128:(tb + 1) * 128,
                        dc4 * 512:(dc4 + 1) * 512], fin[:])

    nc.finalize()
    return nc


_NC_CACHE = {}


def get_nc(gemm_mode=None):
    gemm_mode = gemm_mode or GEMM_MODE
    if gemm_mode not in _NC_CACHE:
        _NC_CACHE[gemm_mode] = _build(gemm_mode)
    return _NC_CACHE[gemm_mode]


def make_in_maps(inputs, gemm_mode=None):
    """Shard full inputs into the 8 per-core input maps."""
    import ml_dtypes

    gemm_mode = gemm_mode or GEMM_MODE
    wnp = ml_dtypes.bfloat16 if gemm_mode == "bf16" else np.float32

    x = np.asarray(inputs["hidden_states"], np.float32).reshape(T, D)
    router_w = np.asarray(inputs["router_w"], np.float32)
    e_bias = np.asarray(inputs["e_bias"], np.float32).reshape(1, E)
    W_gate = np.asarray(inputs["W_gate"])
    W_up = np.asarray(inputs["W_up"])
    W_down = np.asarray(inputs["W_down"])
    shared_gate = np.asarray(inputs["shared_gate"], np.float32)
    shared_up = np.asarray(inputs["shared_up"], np.float32)
    shared_down = np.asarray(inputs["shared_down"], np.float32)

    x_w = np.ascontiguousarray(x).astype(wnp)
    rwT = np.ascontiguousarray(router_w.T.astype(np.float32))
    sgT = np.ascontiguousarray(shared_gate.T).astype(wnp)
    suT = np.ascontiguousarray(shared_up.T).astype(wnp)
    sdT = np.ascontiguousarray(shared_down.T).astype(wnp)

    in_maps = []
    for c in range(NCORES):
        sl = np.zeros((2 * E, 36), np.float32)
        for j in range(EL):
            sl[EL * c + j, j] = 1.0
            sl[E + EL * c + j, 32 + j] = 1.0
        in_maps.append({
            "x_full": x_w,
            "x_own": np.ascontiguousarray(x[TL * c:TL * (c + 1)]),
            "rwT": rwT,
            "ebias": e_bias,
            "sloc": sl,
            "wg": np.ascontiguousarray(W_gate[EL * c:EL * (c + 1)]).astype(wnp),
            "wu": np.ascontiguousarray(W_up[EL * c:EL * (c + 1)]).astype(wnp),
            "wd": np.ascontiguousarray(W_down[EL * c:EL * (c + 1)]).astype(wnp),
            "sgT": sgT,
            "suT": suT,
            "sdT": sdT,
        })
    return in_maps


def kernel(**inputs):
    from concourse.bass_utils import run_bass_kernel_spmd

    nc = get_nc()
    in_maps = make_in_maps(inputs)
    trace = bool(int(os.environ.get("BASS_MOE_TRACE", "0")))
    res = run_bass_kernel_spmd(
        nc, in_maps, core_ids=list(range(NCORES)), trace=trace)
    if trace and res.exec_time_ns is not None:
        print(f"HW exec time: {res.exec_time_ns} ns")
        kernel.last_exec_time_ns = res.exec_time_ns
    out = np.concatenate([res.results[c]["out"] for c in range(NCORES)], axis=0)
    return out.reshape(B, S, D)


kernel.last_exec_time_ns = None



# revision 27
# speedup vs baseline: 1.1119x; 1.1119x over previous
"""DeepseekV3 MoE layer on 8 Trainium2 NeuronCores (Bass/Tile).

Sharding:
  - Router: data-parallel (each core routes its own T/8=512 tokens in fp32 on
    exact logits), then AllGather of per-token (sel-mask, weight) so every
    core knows the full routing.
  - Capacity ranks: per-expert running count over tokens via DVE prefix scan;
    rank <= C survives (matches the reference's stable-sort capacity drop).
  - Routed experts: expert-parallel, 4 experts/core, bf16 GEMMs.  Token rows
    are dma_gather'ed (transpose mode -> [d, slot] layout directly) by
    compacted slot lists (capacity C=160), GEMM'd over exactly 160 slots,
    weighted, and dma_scatter_add'ed (full 4KB rows, bf16) into a [T, D]
    bf16 partial; dropped/invalid slots carry index -1 and are skipped.
  - Combine: ReduceScatter(add, bf16) of partials -> each core owns its
    512-token slice; adds its locally computed shared-expert MLP (kept in
    SBUF) and writes the output slice in fp32.

kernel(**inputs) takes the full unsharded inputs and returns the full
[B, S, D] output.  Self-contained: hardcodes all shapes.
"""

import os
import sys

for _p in ("/opt/trn_rl_repo", "/opt/pypackages"):
    if _p not in sys.path:
        sys.path.insert(0, _p)

import numpy as np

# ---------------------------------------------------------------- constants
B, S, D = 2, 2048, 2048
T = B * S                  # 4096 tokens
I = 1024                   # routed expert intermediate
E = 32                     # routed experts
K = 4                      # experts per token
NG = 8                     # groups
GS = E // NG               # experts per group = 4
TKG = 3                    # top-k groups
ISH = 2048                 # shared expert intermediate (I * n_shared)
SCALE = 2.5
C = 160                    # capacity = ceil(1.25 * T / E)
SB1 = C - 128              # second slot tile rows = 32
NCORES = 8
EL = E // NCORES           # local experts per core = 4
TL = T // NCORES           # local tokens per core = 512

# ablation for profiling: comma list of phases to skip
# {p1router,shared,ag,tables,routed,rs}
ABLATE = frozenset(
    p for p in os.environ.get("BASS_MOE_ABLATE", "").split(",") if p)


# ---------------------------------------------------------------- builder
def _build(ablate: frozenset = frozenset()):
    import concourse.bass as bass
    import concourse.bacc as bacc
    import concourse.mybir as mybir
    import concourse.tile as tile
    from concourse import masks
    from contextlib import ExitStack

    dt = mybir.dt
    Alu = mybir.AluOpType
    Act = mybir.ActivationFunctionType

    f32 = dt.float32
    bf16 = dt.bfloat16

    nc = bacc.Bacc(None, num_devices=NCORES, num_swdge_queues=1)
    groups = [list(range(NCORES))]

    # ---------------- I/O ----------------
    x_full = nc.dram_tensor("x_full", [T, D], bf16, kind="ExternalInput")
    x_own = nc.dram_tensor("x_own", [TL, D], f32, kind="ExternalInput")
    rwT = nc.dram_tensor("rwT", [D, E], f32, kind="ExternalInput")
    ebias = nc.dram_tensor("ebias", [1, E], f32, kind="ExternalInput")
    sloc = nc.dram_tensor("sloc", [2 * E, 36], f32, kind="ExternalInput")
    wgu = nc.dram_tensor("wgu", [EL, D, 2 * I], bf16, kind="ExternalInput")
    wd = nc.dram_tensor("wd", [EL, I, D], bf16, kind="ExternalInput")
    sguT = nc.dram_tensor("sguT", [D, 2 * ISH], bf16, kind="ExternalInput")
    sdT = nc.dram_tensor("sdT", [ISH, D], bf16, kind="ExternalInput")
    out = nc.dram_tensor("out", [TL, D], f32, kind="ExternalOutput")

    # ---------------- internal DRAM ----------------
    selw_own = nc.dram_tensor("selw_own", [TL, 2 * E], f32)
    selw_all = nc.dram_tensor("selw_all", [T, 2 * E], f32, addr_space="Shared")
    partial = nc.dram_tensor("partial", [T, D], bf16)
    rs_out = nc.dram_tensor("rs_out", [TL, D], bf16)
    idx_dram = nc.dram_tensor("idx_dram", [16, EL * 16], dt.int16)
    at_dram = nc.dram_tensor("at_dram", [EL, T], f32)
    nf_dram = nc.dram_tensor("nf_dram", [1, EL], f32)
    aw_dram = nc.dram_tensor("aw_dram", [EL, T], f32)

    DC = D // 128            # 16 d-chunks
    IC = I // 128            # 8  i-chunks
    MC = ISH // 128          # 16 shared-intermediate chunks
    TT = TL // 128           # 4 own-token tiles
    CH = 4                   # routing-table token chunks
    CT = T // CH             # 1024 tokens per chunk

    with tile.TileContext(nc) as tc, ExitStack() as ctx:
        consts = ctx.enter_context(tc.tile_pool(name="consts", bufs=1))
        work = ctx.enter_context(tc.tile_pool(name="work", bufs=2))
        psum_t = ctx.enter_context(
            tc.tile_pool(name="psum_t", bufs=2, space="PSUM"))
        psum_g = ctx.enter_context(
            tc.tile_pool(name="psum_g", bufs=2, space="PSUM"))
        psum_u = ctx.enter_context(
            tc.tile_pool(name="psum_u", bufs=2, space="PSUM"))
        psum_y = ctx.enter_context(
            tc.tile_pool(name="psum_y", bufs=2, space="PSUM"))
        persist = ctx.enter_context(tc.tile_pool(name="persist", bufs=1))
        wstream = ctx.enter_context(tc.tile_pool(name="wstream", bufs=2))

        # ---------------- constants ----------------
        ident = consts.tile([128, 128], f32)
        masks.make_identity(nc, ident[:])

        ebias_b = consts.tile([128, E], f32)
        nc.sync.dma_start(ebias_b[:], ebias[0:1, :].broadcast_to([128, E]))

        negbuf = consts.tile([128, E], f32)
        nc.gpsimd.memset(negbuf[:], -1e30)

        iota16_i = consts.tile([16, 16], dt.int32)
        nc.gpsimd.iota(iota16_i[:], pattern=[[16, 16]], base=0,
                       channel_multiplier=1)
        iota16 = consts.tile([16, 16], f32)
        nc.vector.tensor_copy(iota16[:], iota16_i[:])

        # zero-fill the [T, D] bf16 partial early (overlaps with compute):
        # 8 DMAs of [128, 4, 2048] (8 KB descriptors)
        zt = consts.tile([128, 4, D], bf16)
        nc.gpsimd.memset(zt[:], 0.0)
        pview = partial[:].rearrange("(n p) d -> p n d", p=128)
        for g in range(T // 128 // 4):
            nc.sync.dma_start(pview[:, g * 4:(g + 1) * 4, :], zt[:])

        # ---------------- P1: transpose own tokens -> xT [128, DC, TL] ----
        xtp_cm = tc.tile_pool(name="xtp", bufs=1)
        xtp = xtp_cm.__enter__()
        xT = xtp.tile([128, DC, TL], f32)
        ab_p1 = "p1router" in ablate
        if ab_p1:
            nc.gpsimd.memset(xT[:], 0.0)
        for tt in range(0 if ab_p1 else TT):
            for dc2 in range(DC // 2):
                xtile = work.tile([128, 256], f32, tag="xtile")
                nc.sync.dma_start(
                    xtile[:],
                    x_own[tt * 128:(tt + 1) * 128, dc2 * 256:(dc2 + 1) * 256])
                for h in range(2):
                    dc = dc2 * 2 + h
                    pt = psum_t.tile([128, 128], f32, tag="pt")
                    nc.tensor.transpose(
                        pt[:], xtile[:, h * 128:(h + 1) * 128], ident[:])
                    nc.vector.tensor_copy(
                        xT[:, dc, tt * 128:(tt + 1) * 128], pt[:])
        xTw = xtp.tile([128, DC, TL], bf16)
        for dc in range(DC):
            nc.vector.tensor_copy(xTw[:, dc, :], xT[:, dc, :])

        # ---------------- P2: router on own tokens (fp32/exact) -----------
        rwT_sb = consts.tile([128, DC, E], f32)
        nc.sync.dma_start(
            rwT_sb[:], rwT[:].rearrange("(c p) e -> p c e", p=128))

        for tt in range(0 if ab_p1 else TT):
            ps = psum_t.tile([128, E], f32, tag="pt")
            for dc in range(DC):
                nc.tensor.matmul(
                    ps[:], xT[:, dc, tt * 128:(tt + 1) * 128], rwT_sb[:, dc, :],
                    start=(dc == 0), stop=(dc == DC - 1))
            L = work.tile([128, E], f32, tag="rL")
            nc.vector.tensor_copy(L[:], ps[:])
            Ssig = work.tile([128, E], f32, tag="rS")
            nc.scalar.activation(Ssig[:], ps[:], Act.Sigmoid)
            Sb = work.tile([128, E], f32, tag="rSb")
            nc.vector.tensor_tensor(Sb[:], Ssig[:], ebias_b[:], op=Alu.add)

            # group score = top-2 sum per group = max over pair sums
            Sv = Sb[:].rearrange("p (g i) -> p g i", i=GS)
            gs = work.tile([128, NG], f32, tag="rGS")
            tmp = work.tile([128, NG], f32, tag="rtmp")
            nc.vector.tensor_tensor(gs[:], Sv[:, :, 0], Sv[:, :, 1], op=Alu.add)
            for (a, b) in [(0, 2), (0, 3), (1, 2), (1, 3), (2, 3)]:
                nc.vector.tensor_tensor(
                    tmp[:], Sv[:, :, a], Sv[:, :, b], op=Alu.add)
                nc.vector.tensor_tensor(gs[:], gs[:], tmp[:], op=Alu.max)

            m8g = work.tile([128, 8], f32, tag="rm8g")
            nc.vector.max(m8g[:], gs[:])
            gmask = work.tile([128, NG], f32, tag="rgm")
            nc.vector.tensor_scalar(
                gmask[:], gs[:], m8g[:, TKG - 1:TKG], None, op0=Alu.is_ge)

            emask = work.tile([128, E], f32, tag="rem")
            emv = emask[:].rearrange("p (g i) -> p g i", i=GS)
            for r in range(GS):
                nc.vector.tensor_copy(emv[:, :, r], gmask[:])

            # top-4 experts among unmasked, compared on exact logits
            emask8 = work.tile([128, E], dt.uint8, tag="rem8")
            nc.vector.tensor_copy(emask8[:], emask[:])
            ml = work.tile([128, E], f32, tag="rml")
            nc.vector.tensor_copy(ml[:], negbuf[:])
            nc.vector.copy_predicated(ml[:], emask8[:], L[:])
            m8e = work.tile([128, 8], f32, tag="rm8e")
            nc.vector.max(m8e[:], ml[:])
            sel = work.tile([128, E], f32, tag="rsel")
            nc.vector.tensor_scalar(
                sel[:], ml[:], m8e[:, K - 1:K], None, op0=Alu.is_ge)

            wm = work.tile([128, E], f32, tag="rwm")
            nc.vector.tensor_tensor(wm[:], Ssig[:], sel[:], op=Alu.mult)
            den = work.tile([128, 1], f32, tag="rden")
            nc.vector.tensor_reduce(
                den[:], wm[:], axis=mybir.AxisListType.X, op=Alu.add)
            nc.vector.tensor_scalar(den[:], den[:], 1e-20, None, op0=Alu.add)
            winv = work.tile([128, 1], f32, tag="rwinv")
            nc.vector.reciprocal(winv[:], den[:])

            sw = work.tile([128, 2 * E], f32, tag="rsw")
            nc.vector.tensor_copy(sw[:, 0:E], sel[:])
            nc.vector.tensor_scalar(
                sw[:, E:2 * E], wm[:], winv[:, 0:1], SCALE,
                op0=Alu.mult, op1=Alu.mult)
            nc.sync.dma_start(selw_own[tt * 128:(tt + 1) * 128, :], sw[:])

        # ---------------- P8a: shared expert gate/up (independent) --------
        ab_sh = "shared" in ablate
        HsT = persist.tile([128, MC, TL], bf16)
        for mc in range(0 if ab_sh else MC):
            sgu_t = wstream.tile([128, DC, 256], bf16, tag="wst")
            nc.sync.dma_start(
                sgu_t[:],
                sguT[:].rearrange("(c p) i -> p c i", p=128)
                [:, :, mc * 256:(mc + 1) * 256])
            pg = psum_g.tile([128, TL], f32, tag="pg")
            pu = psum_u.tile([128, TL], f32, tag="pu")
            for dc in range(DC):
                nc.tensor.matmul(
                    pg[:], sgu_t[:, dc, 0:128], xTw[:, dc, :],
                    start=(dc == 0), stop=(dc == DC - 1))
            for dc in range(DC):
                nc.tensor.matmul(
                    pu[:], sgu_t[:, dc, 128:256], xTw[:, dc, :],
                    start=(dc == 0), stop=(dc == DC - 1))
            sig = work.tile([128, TL], f32, tag="ssig")
            nc.scalar.activation(sig[:], pg[:], Act.Sigmoid)
            sil = work.tile([128, TL], bf16, tag="ssil")
            nc.vector.tensor_tensor(sil[:], sig[:], pg[:], op=Alu.mult)
            nc.vector.tensor_tensor(HsT[:, mc, :], sil[:], pu[:], op=Alu.mult)

        xtp_cm.__exit__(None, None, None)

        # ---------------- P3: AllGather routing ----------------
        if "ag" not in ablate:
            nc.gpsimd.collective_compute(
                "AllGather", Alu.bypass, replica_groups=groups,
                ins=[selw_own[:]], outs=[selw_all[:]])

        # ---------------- P4: routing tables (chunked over tokens) --------
        ab_tab = "tables" in ablate
        sloc_sb = consts.tile([64, 36], f32)
        nc.sync.dma_start(sloc_sb[:], sloc[:])

        sgin_t = persist.tile([16, EL, T // 16], f32)
        sgin_w = persist.tile([16, EL, T // 16], f32)
        carry = persist.tile([EL, 1], f32)
        nc.gpsimd.memset(carry[:], 0.0)

        route_cm = tc.tile_pool(name="route", bufs=1)
        route = route_cm.__enter__()
        for q in range(0 if ab_tab else CH):
            selwT_c = route.tile([64, CT // 128, 128], f32, tag="selwT")
            for j in range(CT // 128):
                tt = q * (CT // 128) + j
                swt = work.tile([128, 2 * E], f32, tag="swt")
                nc.sync.dma_start(
                    swt[:], selw_all[tt * 128:(tt + 1) * 128, :])
                pt = psum_t.tile([64, 128], f32, tag="pt")
                nc.tensor.transpose(pt[:], swt[:], ident[:])
                nc.vector.tensor_copy(selwT_c[:, j, :], pt[:])

            SW_sel = route.tile([EL, CT], f32, tag="SWsel")
            SW_w = route.tile([EL, CT], f32, tag="SWw")
            for h in range(CT // 512):
                pswl = psum_g.tile([36, 512], f32, tag="pg")
                nc.tensor.matmul(
                    pswl[:], sloc_sb[:], selwT_c[:, 4 * h:4 * (h + 1), :],
                    start=True, stop=True)
                nc.vector.tensor_copy(
                    SW_sel[:, h * 512:(h + 1) * 512], pswl[0:EL, :])
                nc.vector.tensor_copy(
                    SW_w[:, h * 512:(h + 1) * 512], pswl[32:36, :])

            rank_c = route.tile([EL, CT], f32, tag="rankc")
            nc.vector.tensor_tensor_scan(
                rank_c[:], SW_sel[:], SW_sel[:], carry[:, 0:1],
                op0=Alu.add, op1=Alu.bypass)
            nc.vector.tensor_copy(carry[:], rank_c[:, CT - 1:CT])

            fsel_c = route.tile([EL, CT], f32, tag="fselc")
            nc.vector.tensor_scalar(
                fsel_c[:], rank_c[:], float(C), None, op0=Alu.is_le)
            nc.vector.tensor_tensor(
                fsel_c[:], fsel_c[:], SW_sel[:], op=Alu.mult)

            iota_i = route.tile([EL, CT], dt.int32, tag="iotai")
            nc.gpsimd.iota(iota_i[:], pattern=[[1, CT]], base=1 + q * CT,
                           channel_multiplier=0)
            iota_f = route.tile([EL, CT], f32, tag="iotaf")
            nc.vector.tensor_copy(iota_f[:], iota_i[:])

            At_c = route.tile([EL, CT], f32, tag="Atc")
            nc.vector.tensor_tensor(At_c[:], fsel_c[:], iota_f[:], op=Alu.mult)
            nc.vector.tensor_scalar(At_c[:], At_c[:], 1.0, None,
                                    op0=Alu.subtract)

            fsel8 = route.tile([EL, CT], dt.uint8, tag="fsel8")
            nc.vector.tensor_copy(fsel8[:], fsel_c[:])
            Aw_c = route.tile([EL, CT], f32, tag="Awc")
            nc.gpsimd.memset(Aw_c[:], -1.0)
            nc.vector.copy_predicated(Aw_c[:], fsel8[:], SW_w[:])

            nc.sync.dma_start(at_dram[:, q * CT:(q + 1) * CT], At_c[:])
            nc.sync.dma_start(aw_dram[:, q * CT:(q + 1) * CT], Aw_c[:])

        for e in range(0 if ab_tab else EL):
            nc.sync.dma_start(
                sgin_t[:, e, :],
                at_dram[e].rearrange("(c b) -> b c", b=16))
            nc.sync.dma_start(
                sgin_w[:, e, :],
                aw_dram[e].rearrange("(c b) -> b c", b=16))

        # per-expert compaction -> slot lists + weights.  Invalid slots
        # (beyond num-found) get index 0 / weight 0: the gather/scatter
        # contract requires num_idxs_reg == count(idx >= 0), so dummies
        # point at token 0 and contribute exact zeros.
        idx16s = persist.tile([16, EL * 16], dt.int16)   # 16-row wrapped
        idx16 = persist.tile([128, EL * 16], dt.int16)   # replicated to 128
        w_col = persist.tile([128, 2 * EL], f32)

        sgtoks, sgws = [], []
        for e in range(0 if ab_tab else EL):
            sgtok = work.tile([16, 16], f32, tag=f"sgtok{e}")
            nft = work.tile([1, 1], dt.uint32, tag=f"nft{e}")
            nc.gpsimd.sparse_gather(sgtok[:], sgin_t[:, e, :], num_found=nft[:])
            sgw = work.tile([16, 16], f32, tag=f"sgw{e}")
            nfw = work.tile([1, 1], dt.uint32, tag=f"nfw{e}")
            nc.gpsimd.sparse_gather(sgw[:], sgin_w[:, e, :], num_found=nfw[:])
            nf_f = work.tile([1, 1], f32, tag=f"nf_f{e}")
            nc.vector.tensor_copy(nf_f[:], nft[:])
            nc.sync.dma_start(nf_dram[0:1, e:e + 1], nf_f[:])
            sgtoks.append(sgtok)
            sgws.append(sgw)

        for e in range(0 if ab_tab else EL):
            sgtok, sgw = sgtoks[e], sgws[e]
            nf16 = work.tile([16, 1], f32, tag=f"nf16{e}")
            nc.sync.dma_start(
                nf16[:], nf_dram[0:1, e:e + 1].broadcast_to([16, 1]))
            vm = work.tile([16, 16], f32, tag=f"vm{e}")
            nc.vector.tensor_scalar(
                vm[:], iota16[:], nf16[:, 0:1], None, op0=Alu.is_lt)

            tokm = work.tile([16, 16], f32, tag=f"tokm{e}")
            nc.vector.tensor_tensor(tokm[:], sgtok[:], vm[:], op=Alu.mult)
            nc.vector.tensor_copy(idx16s[:, 16 * e:16 * (e + 1)], tokm[:])

            wsl = work.tile([16, 16], f32, tag=f"wsl{e}")
            nc.vector.tensor_tensor(wsl[:], sgw[:], vm[:], op=Alu.mult)
            ptw = psum_t.tile([16, 16], f32, tag="pt")
            nc.tensor.transpose(ptw[:], wsl[:], ident[:16, :16])
            wt16 = work.tile([16, 16], f32, tag=f"wt16{e}")
            nc.vector.tensor_copy(wt16[:], ptw[:])
            nc.sync.dma_start(w_col[:, 2 * e:2 * e + 1], wt16[0:8, :])
            nc.sync.dma_start(w_col[:, 2 * e + 1:2 * e + 2], wt16[8:16, :])

        # replicate the 16-row wrapped index block to all 128 partitions
        if ab_tab:
            nc.gpsimd.memset(idx16[:], 0)
            nc.gpsimd.memset(w_col[:], 0.0)
        else:
            nc.sync.dma_start(idx_dram[:], idx16s[:])
            for r in range(8):
                nc.sync.dma_start(idx16[16 * r:16 * (r + 1), :], idx_dram[:])

        route_cm.__exit__(None, None, None)

        # ---------------- P5..P7: dispatch + routed expert GEMMs ----------
        with tc.tile_pool(name="dpXPT", bufs=2) as dpXPT, \
                tc.tile_pool(name="dpHT", bufs=2) as dpHT, \
                tc.tile_pool(name="dpWD", bufs=2) as dpWD, \
                tc.tile_pool(name="dpY", bufs=2) as dpY:
            for e in range(0 if "routed" in ablate else EL):
                # gather token rows transposed: XPT[p, dc, slot].  num_idxs
                # must be a multiple of 128 in transpose mode, so gather 256
                # (slots past capacity point at token 0; the GEMMs below
                # only read the first C=160 slots).
                XPT = dpXPT.tile([128, DC, 256], bf16, tag="XPT")
                nc.gpsimd.dma_gather(
                    XPT[:], x_full[:], idx16[:, 16 * e:16 * (e + 1)],
                    256, 256, D, transpose=True, queue_num=0)

                HT = dpHT.tile([128, IC, C], bf16, tag="HT")
                for ic in range(IC):
                    wgu_t = wstream.tile([128, DC, 256], bf16, tag="wst")
                    nc.sync.dma_start(
                        wgu_t[:],
                        wgu[e].rearrange("(c p) i -> p c i", p=128)
                        [:, :, ic * 256:(ic + 1) * 256])
                    pg = psum_g.tile([128, C], f32, tag="pg")
                    pu = psum_u.tile([128, C], f32, tag="pu")
                    for dc in range(DC):
                        nc.tensor.matmul(
                            pg[:], wgu_t[:, dc, 0:128], XPT[:, dc, 0:C],
                            start=(dc == 0), stop=(dc == DC - 1))
                    for dc in range(DC):
                        nc.tensor.matmul(
                            pu[:], wgu_t[:, dc, 128:256], XPT[:, dc, 0:C],
                            start=(dc == 0), stop=(dc == DC - 1))
                    sig = work.tile([128, C], f32, tag="esig")
                    nc.scalar.activation(sig[:], pg[:], Act.Sigmoid)
                    sil = work.tile([128, C], bf16, tag="esil")
                    nc.vector.tensor_tensor(sil[:], sig[:], pg[:], op=Alu.mult)
                    nc.vector.tensor_tensor(
                        HT[:, ic, :], sil[:], pu[:], op=Alu.mult)

                # down proj: accumulate weighted full rows (slot a*128+p at
                # yf[p, a, :]), then one scatter-add of the 160 live slots
                yf = dpY.tile([128, 2, D], bf16, tag="yf")
                for p0 in range(SB1, 128, 32):
                    nc.vector.memset(yf[p0:p0 + 32, 1, :], 0.0)
                for dc4 in range(D // 512):
                    wd_t = dpWD.tile([128, IC, 512], bf16, tag="wst3")
                    nc.sync.dma_start(
                        wd_t[:],
                        wd[e].rearrange("(c p) d -> p c d", p=128)
                        [:, :, dc4 * 512:(dc4 + 1) * 512])
                    py = psum_y.tile([128, 512], f32, tag="py")
                    for ic in range(IC):
                        nc.tensor.matmul(
                            py[:], HT[:, ic, 0:128], wd_t[:, ic, :],
                            start=(ic == 0), stop=(ic == IC - 1))
                    nc.vector.tensor_scalar(
                        yf[:, 0, dc4 * 512:(dc4 + 1) * 512], py[:],
                        w_col[:, 2 * e:2 * e + 1], None, op0=Alu.mult)
                    py2 = psum_y.tile([SB1, 512], f32, tag="py")
                    for ic in range(IC):
                        nc.tensor.matmul(
                            py2[:], HT[:, ic, 128:C], wd_t[:, ic, :],
                            start=(ic == 0), stop=(ic == IC - 1))
                    nc.vector.tensor_scalar(
                        yf[0:SB1, 1, dc4 * 512:(dc4 + 1) * 512], py2[:],
                        w_col[0:SB1, 2 * e + 1:2 * e + 2], None, op0=Alu.mult)
                nc.gpsimd.dma_scatter_add(
                    partial[:], yf[:],
                    idx16[:, 16 * e:16 * e + C // 16], C, C, D, queue_num=0)

        # ---------------- P9: ReduceScatter ----------------
        ab_rs = "rs" in ablate
        if not ab_rs:
            nc.gpsimd.collective_compute(
                "ReduceScatter", Alu.add, replica_groups=groups,
                ins=[partial[:]], outs=[rs_out[:]])

        # ---------------- P8b: shared expert down proj -> SBUF ------------
        shr_sb = persist.tile([128, TT, D // 512, 512], f32)
        if ab_sh:
            nc.gpsimd.memset(shr_sb[:], 0.0)
        with tc.tile_pool(name="sdpool", bufs=2) as sdpool:
            for dc4 in range(0 if ab_sh else D // 512):
                sd_t = sdpool.tile([128, MC, 512], bf16, tag="wsd")
                nc.sync.dma_start(
                    sd_t[:],
                    sdT[:].rearrange("(c p) d -> p c d", p=128)
                    [:, :, dc4 * 512:(dc4 + 1) * 512])
                for tb in range(TT):
                    po = psum_y.tile([128, 512], f32, tag="py")
                    for mc in range(MC):
                        nc.tensor.matmul(
                            po[:], HsT[:, mc, tb * 128:(tb + 1) * 128],
                            sd_t[:, mc, :],
                            start=(mc == 0), stop=(mc == MC - 1))
                    nc.vector.tensor_copy(shr_sb[:, tb, dc4, :], po[:])

        # ---------------- P10: final add ----------------
        rs_src = partial if ab_rs else rs_out
        for tb in range(TT):
            for dc4 in range(D // 512):
                rst = work.tile([128, 512], bf16, tag="rst")
                nc.sync.dma_start(
                    rst[:], rs_src[tb * 128:(tb + 1) * 128,
                                   dc4 * 512:(dc4 + 1) * 512])
                fin = work.tile([128, 512], f32, tag="fin")
                nc.vector.tensor_tensor(
                    fin[:], shr_sb[:, tb, dc4, :], rst[:], op=Alu.add)
                nc.sync.dma_start(
                    out[tb * 128:(tb + 1) * 128,
                        dc4 * 512:(dc4 + 1) * 512], fin[:])

    nc.finalize()
    return nc


_NC_CACHE = {}


def get_nc():
    key = ABLATE
    if key not in _NC_CACHE:
        _NC_CACHE[key] = _build(ABLATE)
    return _NC_CACHE[key]


def make_in_maps(inputs):
    """Shard full inputs into the 8 per-core input maps."""
    import ml_dtypes

    bf16 = ml_dtypes.bfloat16

    x = np.asarray(inputs["hidden_states"], np.float32).reshape(T, D)
    router_w = np.asarray(inputs["router_w"], np.float32)
    e_bias = np.asarray(inputs["e_bias"], np.float32).reshape(1, E)
    W_gate = np.asarray(inputs["W_gate"], np.float32)
    W_up = np.asarray(inputs["W_up"], np.float32)
    W_down = np.asarray(inputs["W_down"], np.float32)
    shared_gate = np.asarray(inputs["shared_gate"], np.float32)
    shared_up = np.asarray(inputs["shared_up"], np.float32)
    shared_down = np.asarray(inputs["shared_down"], np.float32)

    x_w = np.ascontiguousarray(x).astype(bf16)
    rwT = np.ascontiguousarray(router_w.T.astype(np.float32))
    # interleave gate/up in 128-col blocks: one DMA feeds both GEMMs
    wgu = np.stack([W_gate.reshape(E, D, I // 128, 128),
                    W_up.reshape(E, D, I // 128, 128)],
                   axis=3).reshape(E, D, 2 * I).astype(bf16)
    sgT = np.ascontiguousarray(shared_gate.T)
    suT = np.ascontiguousarray(shared_up.T)
    sguT = np.stack([sgT.reshape(D, ISH // 128, 128),
                     suT.reshape(D, ISH // 128, 128)],
                    axis=2).reshape(D, 2 * ISH).astype(bf16)
    sdT = np.ascontiguousarray(shared_down.T).astype(bf16)
    wd_b = W_down.astype(bf16)

    in_maps = []
    for c in range(NCORES):
        sl = np.zeros((2 * E, 36), np.float32)
        for j in range(EL):
            sl[EL * c + j, j] = 1.0
            sl[E + EL * c + j, 32 + j] = 1.0
        in_maps.append({
            "x_full": x_w,
            "x_own": np.ascontiguousarray(x[TL * c:TL * (c + 1)]),
            "rwT": rwT,
            "ebias": e_bias,
            "sloc": sl,
            "wgu": np.ascontiguousarray(wgu[EL * c:EL * (c + 1)]),
            "wd": np.ascontiguousarray(wd_b[EL * c:EL * (c + 1)]),
            "sguT": sguT,
            "sdT": sdT,
        })
    return in_maps


def kernel(**inputs):
    from concourse.bass_utils import run_bass_kernel_spmd

    nc = get_nc()
    in_maps = make_in_maps(inputs)
    trace = bool(int(os.environ.get("BASS_MOE_TRACE", "0")))
    res = run_bass_kernel_spmd(
        nc, in_maps, core_ids=list(range(NCORES)), trace=trace)
    if trace and res.exec_time_ns is not None:
        print(f"HW exec time: {res.exec_time_ns} ns")
        kernel.last_exec_time_ns = res.exec_time_ns
    out = np.concatenate([res.results[c]["out"] for c in range(NCORES)], axis=0)
    return out.reshape(B, S, D)


kernel.last_exec_time_ns = None


# revision 64
# speedup vs baseline: 1.1709x; 1.0530x over previous
"""DeepseekV3 MoE layer on 8 Trainium2 NeuronCores (Bass/Tile).

Sharding:
  - Router: data-parallel (each core routes its own T/8=512 tokens in fp32 on
    exact logits), then AllGather of per-token (sel-mask, weight) so every
    core knows the full routing.
  - Capacity ranks: per-expert running count over tokens via DVE prefix scan;
    rank <= C survives (matches the reference's stable-sort capacity drop).
  - Routed experts: expert-parallel, 4 experts/core, bf16 GEMMs.  Token rows
    are dma_gather'ed (transpose mode -> [d, slot] layout directly) by
    compacted slot lists (capacity C=160), GEMM'd over exactly 160 slots,
    weighted, and dma_scatter_add'ed (full 4KB rows, bf16) into a [T, D]
    bf16 partial; dropped/invalid slots carry index -1 and are skipped.
  - Combine: ReduceScatter(add, bf16) of partials -> each core owns its
    512-token slice; adds its locally computed shared-expert MLP (kept in
    SBUF) and writes the output slice in fp32.

kernel(**inputs) takes the full unsharded inputs and returns the full
[B, S, D] output.  Self-contained: hardcodes all shapes.
"""

import os
import sys

for _p in ("/opt/trn_rl_repo", "/opt/pypackages"):
    if _p not in sys.path:
        sys.path.insert(0, _p)

import numpy as np

# ---------------------------------------------------------------- constants
B, S, D = 2, 2048, 2048
T = B * S                  # 4096 tokens
I = 1024                   # routed expert intermediate
E = 32                     # routed experts
K = 4                      # experts per token
NG = 8                     # groups
GS = E // NG               # experts per group = 4
TKG = 3                    # top-k groups
ISH = 2048                 # shared expert intermediate (I * n_shared)
SCALE = 2.5
C = 160                    # capacity = ceil(1.25 * T / E)
SB1 = C - 128              # second slot tile rows = 32
NCORES = 8
EL = E // NCORES           # local experts per core = 4
TL = T // NCORES           # local tokens per core = 512

# ablation for profiling: comma list of phases to skip
# {p1router,shared,ag,tables,routed,rs}
ABLATE = frozenset(
    p for p in os.environ.get("BASS_MOE_ABLATE", "").split(",") if p)


# ---------------------------------------------------------------- builder
def _build(ablate: frozenset = frozenset()):
    import concourse.bass as bass
    import concourse.bacc as bacc
    import concourse.mybir as mybir
    import concourse.tile as tile
    from concourse import masks
    from contextlib import ExitStack

    dt = mybir.dt
    Alu = mybir.AluOpType
    Act = mybir.ActivationFunctionType

    f32 = dt.float32
    bf16 = dt.bfloat16

    nc = bacc.Bacc(None, num_devices=NCORES, num_swdge_queues=4)
    groups = [list(range(NCORES))]

    # ---------------- I/O ----------------
    x_full = nc.dram_tensor("x_full", [T, D], bf16, kind="ExternalInput")
    x_own = nc.dram_tensor("x_own", [TL, D], f32, kind="ExternalInput")
    rwT = nc.dram_tensor("rwT", [D, E], f32, kind="ExternalInput")
    ebias = nc.dram_tensor("ebias", [1, E], f32, kind="ExternalInput")
    sloc = nc.dram_tensor("sloc", [2 * E, 36], f32, kind="ExternalInput")
    wgu = nc.dram_tensor("wgu", [EL, D, 2 * I], bf16, kind="ExternalInput")
    wd = nc.dram_tensor("wd", [EL, I, D], bf16, kind="ExternalInput")
    sguT = nc.dram_tensor("sguT", [D, 2 * ISH], bf16, kind="ExternalInput")
    sdT = nc.dram_tensor("sdT", [ISH, D], bf16, kind="ExternalInput")
    out = nc.dram_tensor("out", [TL, D], f32, kind="ExternalOutput")

    # ---------------- internal DRAM ----------------
    selw_own = nc.dram_tensor("selw_own", [TL, 2 * E], f32)
    selw_all = nc.dram_tensor("selw_all", [T, 2 * E], f32, addr_space="Shared")
    partial = nc.dram_tensor("partial", [T, D], bf16)
    rs_out = nc.dram_tensor("rs_out", [TL, D], bf16)
    at_dram = nc.dram_tensor("at_dram", [EL, T], f32)
    aw_dram = nc.dram_tensor("aw_dram", [EL, T], f32)

    DC = D // 128            # 16 d-chunks
    IC = I // 128            # 8  i-chunks
    MC = ISH // 128          # 16 shared-intermediate chunks
    TT = TL // 128           # 4 own-token tiles
    CH = 4                   # routing-table token chunks
    CT = T // CH             # 1024 tokens per chunk

    with tile.TileContext(nc) as tc, ExitStack() as ctx:
        consts = ctx.enter_context(tc.tile_pool(name="consts", bufs=1))
        work = ctx.enter_context(tc.tile_pool(name="work", bufs=2))
        psum_t = ctx.enter_context(
            tc.tile_pool(name="psum_t", bufs=2, space="PSUM"))
        psum_g = ctx.enter_context(
            tc.tile_pool(name="psum_g", bufs=2, space="PSUM"))
        psum_u = ctx.enter_context(
            tc.tile_pool(name="psum_u", bufs=2, space="PSUM"))
        psum_y = ctx.enter_context(
            tc.tile_pool(name="psum_y", bufs=2, space="PSUM"))
        persist = ctx.enter_context(tc.tile_pool(name="persist", bufs=1))
        wstream = ctx.enter_context(tc.tile_pool(name="wstream", bufs=2))

        # ---------------- constants ----------------
        ident = consts.tile([128, 128], f32)
        masks.make_identity(nc, ident[:])
        identb = consts.tile([128, 128], bf16)
        nc.vector.tensor_copy(identb[:], ident[:])

        ebias_b = consts.tile([128, E], f32)
        nc.sync.dma_start(ebias_b[:], ebias[0:1, :].broadcast_to([128, E]))

        negbuf = consts.tile([128, E], f32)
        nc.gpsimd.memset(negbuf[:], -1e30)

        iota16_i = consts.tile([16, 16], dt.int32)
        nc.gpsimd.iota(iota16_i[:], pattern=[[16, 16]], base=0,
                       channel_multiplier=1)
        iota16 = consts.tile([16, 16], f32)
        nc.vector.tensor_copy(iota16[:], iota16_i[:])

        ones16 = consts.tile([1, 16], f32)
        nc.gpsimd.memset(ones16[:], 1.0)

        # zero-fill the [T, D] bf16 partial early (overlaps with compute):
        # 8 DMAs of [128, 4, 2048] (8 KB descriptors)
        zt = consts.tile([128, 2, D], bf16)
        nc.gpsimd.memset(zt[:], 0.0)
        pview = partial[:].rearrange("(n p) d -> p n d", p=128)
        for g in range(T // 128 // 2):
            nc.sync.dma_start(pview[:, g * 2:(g + 1) * 2, :], zt[:])

        # ---------------- P1: transpose own tokens -> xT [128, DC, TL] ----
        xtp_cm = tc.tile_pool(name="xtp", bufs=1)
        xtp = xtp_cm.__enter__()
        xT = xtp.tile([128, DC, TL], f32)
        ab_p1 = "p1router" in ablate
        if ab_p1:
            nc.gpsimd.memset(xT[:], 0.0)
        for tt in range(0 if ab_p1 else TT):
            for dh in range(2):
                xtile = work.tile([128, D // 2], f32, tag="xtile")
                nc.sync.dma_start(
                    xtile[:],
                    x_own[tt * 128:(tt + 1) * 128,
                          dh * (D // 2):(dh + 1) * (D // 2)])
                for dcl in range(DC // 2):
                    dc = dh * (DC // 2) + dcl
                    pt = psum_t.tile([128, 128], f32, tag="pt")
                    nc.tensor.transpose(
                        pt[:], xtile[:, dcl * 128:(dcl + 1) * 128], ident[:])
                    nc.vector.tensor_copy(
                        xT[:, dc, tt * 128:(tt + 1) * 128], pt[:])
        xTw = xtp.tile([128, DC, TL], bf16)
        for dc in range(DC):
            nc.vector.tensor_copy(xTw[:, dc, :], xT[:, dc, :])

        # ---------------- P2: router on own tokens (fp32/exact) -----------
        rwT_sb = consts.tile([128, DC, E], f32)
        nc.sync.dma_start(
            rwT_sb[:], rwT[:].rearrange("(c p) e -> p c e", p=128))

        for tt in range(0 if ab_p1 else TT):
            ps = psum_t.tile([128, E], f32, tag="pt")
            for dc in range(DC):
                nc.tensor.matmul(
                    ps[:], xT[:, dc, tt * 128:(tt + 1) * 128], rwT_sb[:, dc, :],
                    start=(dc == 0), stop=(dc == DC - 1))
            L = work.tile([128, E], f32, tag="rL")
            nc.vector.tensor_copy(L[:], ps[:])
            Ssig = work.tile([128, E], f32, tag="rS")
            nc.scalar.activation(Ssig[:], ps[:], Act.Sigmoid)
            Sb = work.tile([128, E], f32, tag="rSb")
            nc.vector.tensor_tensor(Sb[:], Ssig[:], ebias_b[:], op=Alu.add)

            # group score = top-2 sum per group = max over pair sums
            Sv = Sb[:].rearrange("p (g i) -> p g i", i=GS)
            gs = work.tile([128, NG], f32, tag="rGS")
            tmp = work.tile([128, NG], f32, tag="rtmp")
            nc.vector.tensor_tensor(gs[:], Sv[:, :, 0], Sv[:, :, 1], op=Alu.add)
            for (a, b) in [(0, 2), (0, 3), (1, 2), (1, 3), (2, 3)]:
                nc.vector.tensor_tensor(
                    tmp[:], Sv[:, :, a], Sv[:, :, b], op=Alu.add)
                nc.vector.tensor_tensor(gs[:], gs[:], tmp[:], op=Alu.max)

            m8g = work.tile([128, 8], f32, tag="rm8g")
            nc.vector.max(m8g[:], gs[:])
            gmask = work.tile([128, NG], f32, tag="rgm")
            nc.vector.tensor_scalar(
                gmask[:], gs[:], m8g[:, TKG - 1:TKG], None, op0=Alu.is_ge)

            emask = work.tile([128, E], f32, tag="rem")
            emv = emask[:].rearrange("p (g i) -> p g i", i=GS)
            for r in range(GS):
                nc.vector.tensor_copy(emv[:, :, r], gmask[:])

            # top-4 experts among unmasked, compared on exact logits
            emask8 = work.tile([128, E], dt.uint8, tag="rem8")
            nc.vector.tensor_copy(emask8[:], emask[:])
            ml = work.tile([128, E], f32, tag="rml")
            nc.vector.tensor_copy(ml[:], negbuf[:])
            nc.vector.copy_predicated(ml[:], emask8[:], L[:])
            m8e = work.tile([128, 8], f32, tag="rm8e")
            nc.vector.max(m8e[:], ml[:])
            sel = work.tile([128, E], f32, tag="rsel")
            nc.vector.tensor_scalar(
                sel[:], ml[:], m8e[:, K - 1:K], None, op0=Alu.is_ge)

            wm = work.tile([128, E], f32, tag="rwm")
            nc.vector.tensor_tensor(wm[:], Ssig[:], sel[:], op=Alu.mult)
            den = work.tile([128, 1], f32, tag="rden")
            nc.vector.tensor_reduce(
                den[:], wm[:], axis=mybir.AxisListType.X, op=Alu.add)
            nc.vector.tensor_scalar(den[:], den[:], 1e-20, None, op0=Alu.add)
            winv = work.tile([128, 1], f32, tag="rwinv")
            nc.vector.reciprocal(winv[:], den[:])

            sw = work.tile([128, 2 * E], f32, tag="rsw")
            nc.vector.tensor_copy(sw[:, 0:E], sel[:])
            nc.vector.tensor_scalar(
                sw[:, E:2 * E], wm[:], winv[:, 0:1], SCALE,
                op0=Alu.mult, op1=Alu.mult)
            nc.sync.dma_start(selw_own[tt * 128:(tt + 1) * 128, :], sw[:])

        # ---------------- P8a: shared expert gate/up (independent) --------
        ab_sh = "shared" in ablate
        HsT = persist.tile([128, MC, TL], bf16)
        for mc in range(0 if ab_sh else MC):
            sgu_t = wstream.tile([128, DC, 256], bf16, tag="wst")
            nc.scalar.dma_start(
                sgu_t[:],
                sguT[:].rearrange("(c p) i -> p c i", p=128)
                [:, :, mc * 256:(mc + 1) * 256])
            pg = psum_g.tile([128, TL], f32, tag="pg")
            pu = psum_u.tile([128, TL], f32, tag="pu")
            for dc in range(DC):
                nc.tensor.matmul(
                    pg[:], sgu_t[:, dc, 0:128], xTw[:, dc, :],
                    start=(dc == 0), stop=(dc == DC - 1))
            for dc in range(DC):
                nc.tensor.matmul(
                    pu[:], sgu_t[:, dc, 128:256], xTw[:, dc, :],
                    start=(dc == 0), stop=(dc == DC - 1))
            sig = work.tile([128, TL], f32, tag="ssig")
            nc.scalar.activation(sig[:], pg[:], Act.Sigmoid)
            sil = work.tile([128, TL], bf16, tag="ssil")
            nc.vector.tensor_tensor(sil[:], sig[:], pg[:], op=Alu.mult)
            nc.vector.tensor_tensor(HsT[:, mc, :], sil[:], pu[:], op=Alu.mult)

        xtp_cm.__exit__(None, None, None)

        # ---------------- P3: AllGather routing ----------------
        if "ag" not in ablate:
            nc.gpsimd.collective_compute(
                "AllGather", Alu.bypass, replica_groups=groups,
                ins=[selw_own[:]], outs=[selw_all[:]])

        # ---------------- P4: routing tables (chunked over tokens) --------
        ab_tab = "tables" in ablate
        sloc_sb = consts.tile([64, 36], f32)
        nc.sync.dma_start(sloc_sb[:], sloc[:])

        sgin_t = persist.tile([16, EL, T // 16], f32)
        sgin_w = persist.tile([16, EL, T // 16], f32)
        carry = persist.tile([EL, 1], f32)
        nc.gpsimd.memset(carry[:], 0.0)

        route_cm = tc.tile_pool(name="route", bufs=1)
        route = route_cm.__enter__()
        for q in range(0 if ab_tab else CH):
            selwT_c = route.tile([64, CT // 128, 128], f32, tag="selwT")
            swt = work.tile([128, CT // 128, 2 * E], f32, tag="swt")
            nc.sync.dma_start(
                swt[:],
                selw_all[q * CT:(q + 1) * CT, :]
                .rearrange("(j p) e -> p j e", p=128))
            for j in range(CT // 128):
                pt = psum_t.tile([64, 128], f32, tag="pt")
                nc.tensor.transpose(pt[:], swt[:, j, :], ident[:])
                nc.vector.tensor_copy(selwT_c[:, j, :], pt[:])

            SW_sel = route.tile([EL, CT], f32, tag="SWsel")
            SW_w = route.tile([EL, CT], f32, tag="SWw")
            for h in range(CT // 512):
                pswl = psum_g.tile([36, 512], f32, tag="pg")
                nc.tensor.matmul(
                    pswl[:], sloc_sb[:], selwT_c[:, 4 * h:4 * (h + 1), :],
                    start=True, stop=True)
                nc.vector.tensor_copy(
                    SW_sel[:, h * 512:(h + 1) * 512], pswl[0:EL, :])
                nc.vector.tensor_copy(
                    SW_w[:, h * 512:(h + 1) * 512], pswl[32:36, :])

            rank_c = route.tile([EL, CT], f32, tag="rankc")
            nc.vector.tensor_tensor_scan(
                rank_c[:], SW_sel[:], SW_sel[:], carry[:, 0:1],
                op0=Alu.add, op1=Alu.bypass)
            nc.vector.tensor_copy(carry[:], rank_c[:, CT - 1:CT])

            fsel_c = route.tile([EL, CT], f32, tag="fselc")
            nc.vector.tensor_scalar(
                fsel_c[:], rank_c[:], float(C), None, op0=Alu.is_le)
            nc.vector.tensor_tensor(
                fsel_c[:], fsel_c[:], SW_sel[:], op=Alu.mult)

            iota_i = route.tile([EL, CT], dt.int32, tag="iotai")
            nc.gpsimd.iota(iota_i[:], pattern=[[1, CT]], base=1 + q * CT,
                           channel_multiplier=0)
            iota_f = route.tile([EL, CT], f32, tag="iotaf")
            nc.vector.tensor_copy(iota_f[:], iota_i[:])

            At_c = route.tile([EL, CT], f32, tag="Atc")
            nc.vector.tensor_tensor(At_c[:], fsel_c[:], iota_f[:], op=Alu.mult)
            nc.vector.tensor_scalar(At_c[:], At_c[:], 1.0, None,
                                    op0=Alu.subtract)

            # Aw = SW_w * fsel + fsel - 1  (weight if selected, else -1)
            Aw_c = route.tile([EL, CT], f32, tag="Awc")
            nc.vector.tensor_tensor(Aw_c[:], SW_w[:], fsel_c[:], op=Alu.mult)
            nc.vector.tensor_tensor(Aw_c[:], Aw_c[:], fsel_c[:], op=Alu.add)
            nc.vector.tensor_scalar(Aw_c[:], Aw_c[:], 1.0, None,
                                    op0=Alu.subtract)

            nc.sync.dma_start(at_dram[:, q * CT:(q + 1) * CT], At_c[:])
            nc.sync.dma_start(aw_dram[:, q * CT:(q + 1) * CT], Aw_c[:])

        for e in range(0 if ab_tab else EL):
            nc.sync.dma_start(
                sgin_t[:, e, :],
                at_dram[e].rearrange("(c b) -> b c", b=16))
            nc.sync.dma_start(
                sgin_w[:, e, :],
                aw_dram[e].rearrange("(c b) -> b c", b=16))

        # per-expert compaction -> slot lists + weights.  Invalid slots
        # (beyond num-found) get index 0 / weight 0: the gather/scatter
        # contract requires num_idxs_reg == count(idx >= 0), so dummies
        # point at token 0 and contribute exact zeros.
        idx16s = persist.tile([16, EL * 16], dt.int16)   # 16-row wrapped
        idx16 = persist.tile([128, EL * 16], dt.int16)   # replicated to 128
        w_col = persist.tile([128, 2 * EL], f32)

        sgtoks, sgws, nfs = [], [], []
        for e in range(0 if ab_tab else EL):
            sgtok = work.tile([16, 16], f32, tag=f"sgtok{e}")
            nft = work.tile([1, 1], dt.uint32, tag=f"nft{e}")
            nc.gpsimd.sparse_gather(sgtok[:], sgin_t[:, e, :], num_found=nft[:])
            sgw = work.tile([16, 16], f32, tag=f"sgw{e}")
            nfw = work.tile([1, 1], dt.uint32, tag=f"nfw{e}")
            nc.gpsimd.sparse_gather(sgw[:], sgin_w[:, e, :], num_found=nfw[:])
            nf_f = work.tile([1, 1], f32, tag=f"nf_f{e}")
            nc.vector.tensor_copy(nf_f[:], nft[:])
            sgtoks.append(sgtok)
            sgws.append(sgw)
            nfs.append(nf_f)

        for e in range(0 if ab_tab else EL):
            sgtok, sgw = sgtoks[e], sgws[e]
            # broadcast num-found to 16 partitions via the PE array
            pnf = psum_t.tile([16, 1], f32, tag="pt")
            nc.tensor.matmul(pnf[:], ones16[:], nfs[e][:],
                             start=True, stop=True)
            nf16 = work.tile([16, 1], f32, tag=f"nf16{e}")
            nc.vector.tensor_copy(nf16[:], pnf[:])
            vm = work.tile([16, 16], f32, tag=f"vm{e}")
            nc.vector.tensor_scalar(
                vm[:], iota16[:], nf16[:, 0:1], None, op0=Alu.is_lt)

            tokm = work.tile([16, 16], f32, tag=f"tokm{e}")
            nc.vector.tensor_tensor(tokm[:], sgtok[:], vm[:], op=Alu.mult)
            nc.vector.tensor_copy(idx16s[:, 16 * e:16 * (e + 1)], tokm[:])

            wsl = work.tile([16, 16], f32, tag=f"wsl{e}")
            nc.vector.tensor_tensor(wsl[:], sgw[:], vm[:], op=Alu.mult)
            ptw = psum_t.tile([16, 16], f32, tag="pt")
            nc.tensor.transpose(ptw[:], wsl[:], ident[:16, :16])
            wt16 = work.tile([16, 16], f32, tag=f"wt16{e}")
            nc.vector.tensor_copy(wt16[:], ptw[:])
            nc.sync.dma_start(w_col[:, 2 * e:2 * e + 1], wt16[0:8, :])
            nc.sync.dma_start(w_col[:, 2 * e + 1:2 * e + 2], wt16[8:16, :])

        # replicate the 16-row wrapped index block to all 128 partitions
        # (SBUF -> SBUF partition-offset copies)
        if ab_tab:
            nc.gpsimd.memset(idx16[:], 0)
            nc.gpsimd.memset(w_col[:], 0.0)
        else:
            for r in range(8):
                nc.sync.dma_start(idx16[16 * r:16 * (r + 1), :], idx16s[:])

        route_cm.__exit__(None, None, None)

        # ---------------- P5..P7: dispatch + routed expert GEMMs ----------
        with tc.tile_pool(name="dpXP", bufs=2) as dpXP, \
                tc.tile_pool(name="dpXPT", bufs=2) as dpXPT, \
                tc.tile_pool(name="dpHT", bufs=2) as dpHT, \
                tc.tile_pool(name="dpWD", bufs=2) as dpWD, \
                tc.tile_pool(name="dpY", bufs=2) as dpY:
            for e in range(0 if "routed" in ablate else EL):
                # gather token rows (4 KB descriptors; slots past capacity
                # point at token 0), then xbar-transpose the C=160 live
                # slots to XPT[p, dc, slot] off the PE/DVE engines
                XP = dpXP.tile([128, 2, D], bf16, tag="XP")
                nc.gpsimd.dma_gather(
                    XP[:], x_full[:], idx16[:, 16 * e:16 * (e + 1)],
                    256, 256, D, queue_num=e % 2)
                XPT = dpXPT.tile([128, DC, C], bf16, tag="XPT")
                nc.scalar.dma_start_transpose(
                    XPT[:, :, 0:128], XP[:, 0, :])
                nc.scalar.dma_start_transpose(
                    XPT[:, :, 128:C], XP[0:SB1, 1, :])

                HT = dpHT.tile([128, IC, C], bf16, tag="HT")
                for ic in range(IC):
                    wgu_t = wstream.tile([128, DC, 256], bf16, tag="wst")
                    nc.scalar.dma_start(
                        wgu_t[:],
                        wgu[e].rearrange("(c p) i -> p c i", p=128)
                        [:, :, ic * 256:(ic + 1) * 256])
                    pg = psum_g.tile([128, C], f32, tag="pg")
                    pu = psum_u.tile([128, C], f32, tag="pu")
                    for dc in range(DC):
                        nc.tensor.matmul(
                            pg[:], wgu_t[:, dc, 0:128], XPT[:, dc, 0:C],
                            start=(dc == 0), stop=(dc == DC - 1))
                    for dc in range(DC):
                        nc.tensor.matmul(
                            pu[:], wgu_t[:, dc, 128:256], XPT[:, dc, 0:C],
                            start=(dc == 0), stop=(dc == DC - 1))
                    sig = work.tile([128, C], f32, tag="esig")
                    nc.scalar.activation(sig[:], pg[:], Act.Sigmoid)
                    sil = work.tile([128, C], bf16, tag="esil")
                    nc.vector.tensor_tensor(sil[:], sig[:], pg[:],
                                            op=Alu.mult)
                    nc.vector.tensor_tensor(
                        HT[:, ic, :], sil[:], pu[:], op=Alu.mult)

                # down proj: accumulate weighted full rows (slot a*128+p at
                # yf[p, a, :]), then one scatter-add of the 160 live slots
                yf = dpY.tile([128, 2, D], bf16, tag="yf")
                for p0 in range(SB1, 128, 32):
                    nc.vector.memset(yf[p0:p0 + 32, 1, :], 0.0)
                for dc4 in range(D // 512):
                    wd_t = dpWD.tile([128, IC, 512], bf16, tag="wst3")
                    nc.scalar.dma_start(
                        wd_t[:],
                        wd[e].rearrange("(c p) d -> p c d", p=128)
                        [:, :, dc4 * 512:(dc4 + 1) * 512])
                    py = psum_y.tile([128, 512], f32, tag="py")
                    for ic in range(IC):
                        nc.tensor.matmul(
                            py[:], HT[:, ic, 0:128], wd_t[:, ic, :],
                            start=(ic == 0), stop=(ic == IC - 1))
                    nc.vector.tensor_scalar(
                        yf[:, 0, dc4 * 512:(dc4 + 1) * 512], py[:],
                        w_col[:, 2 * e:2 * e + 1], None, op0=Alu.mult)
                    py2 = psum_y.tile([SB1, 512], f32, tag="py")
                    for ic in range(IC):
                        nc.tensor.matmul(
                            py2[:], HT[:, ic, 128:C], wd_t[:, ic, :],
                            start=(ic == 0), stop=(ic == IC - 1))
                    nc.vector.tensor_scalar(
                        yf[0:SB1, 1, dc4 * 512:(dc4 + 1) * 512], py2[:],
                        w_col[0:SB1, 2 * e + 1:2 * e + 2], None,
                        op0=Alu.mult)
                nc.gpsimd.dma_scatter_add(
                    partial[:], yf[:],
                    idx16[:, 16 * e:16 * e + C // 16], C, C, D,
                    queue_num=2 + e % 2)

        # ---------------- P9: ReduceScatter ----------------
        ab_rs = "rs" in ablate
        if not ab_rs:
            nc.gpsimd.collective_compute(
                "ReduceScatter", Alu.add, replica_groups=groups,
                ins=[partial[:]], outs=[rs_out[:]])

        # ---------------- P8b: shared expert down proj -> SBUF ------------
        shr_sb = persist.tile([128, TT, D // 512, 512], f32)
        if ab_sh:
            nc.gpsimd.memset(shr_sb[:], 0.0)
        with tc.tile_pool(name="sdpool", bufs=2) as sdpool:
            for dc4 in range(0 if ab_sh else D // 512):
                sd_t = sdpool.tile([128, MC, 512], bf16, tag="wsd")
                nc.sync.dma_start(
                    sd_t[:],
                    sdT[:].rearrange("(c p) d -> p c d", p=128)
                    [:, :, dc4 * 512:(dc4 + 1) * 512])
                for tb in range(TT):
                    po = psum_y.tile([128, 512], f32, tag="py")
                    for mc in range(MC):
                        nc.tensor.matmul(
                            po[:], HsT[:, mc, tb * 128:(tb + 1) * 128],
                            sd_t[:, mc, :],
                            start=(mc == 0), stop=(mc == MC - 1))
                    nc.vector.tensor_copy(shr_sb[:, tb, dc4, :], po[:])

        # ---------------- P10: final add ----------------
        rs_src = partial if ab_rs else rs_out
        for tb in range(TT):
            rst = work.tile([128, D], bf16, tag="rst")
            nc.sync.dma_start(rst[:], rs_src[tb * 128:(tb + 1) * 128, :])
            sslice = shr_sb[:, tb, :, :].rearrange("p a b -> p (a b)")
            nc.vector.tensor_tensor(sslice, sslice, rst[:], op=Alu.add)
            nc.sync.dma_start(out[tb * 128:(tb + 1) * 128, :], sslice)

    nc.finalize()
    return nc


_NC_CACHE = {}


def get_nc():
    key = ABLATE
    if key not in _NC_CACHE:
        _NC_CACHE[key] = _build(ABLATE)
    return _NC_CACHE[key]


def make_in_maps(inputs):
    """Shard full inputs into the 8 per-core input maps."""
    import ml_dtypes

    bf16 = ml_dtypes.bfloat16

    x = np.asarray(inputs["hidden_states"], np.float32).reshape(T, D)
    router_w = np.asarray(inputs["router_w"], np.float32)
    e_bias = np.asarray(inputs["e_bias"], np.float32).reshape(1, E)
    W_gate = np.asarray(inputs["W_gate"], np.float32)
    W_up = np.asarray(inputs["W_up"], np.float32)
    W_down = np.asarray(inputs["W_down"], np.float32)
    shared_gate = np.asarray(inputs["shared_gate"], np.float32)
    shared_up = np.asarray(inputs["shared_up"], np.float32)
    shared_down = np.asarray(inputs["shared_down"], np.float32)

    x_w = np.ascontiguousarray(x).astype(bf16)
    rwT = np.ascontiguousarray(router_w.T.astype(np.float32))
    # interleave gate/up in 128-col blocks: one DMA feeds both GEMMs
    wgu = np.stack([W_gate.reshape(E, D, I // 128, 128),
                    W_up.reshape(E, D, I // 128, 128)],
                   axis=3).reshape(E, D, 2 * I).astype(bf16)
    sgT = np.ascontiguousarray(shared_gate.T)
    suT = np.ascontiguousarray(shared_up.T)
    sguT = np.stack([sgT.reshape(D, ISH // 128, 128),
                     suT.reshape(D, ISH // 128, 128)],
                    axis=2).reshape(D, 2 * ISH).astype(bf16)
    sdT = np.ascontiguousarray(shared_down.T).astype(bf16)
    wd_b = W_down.astype(bf16)

    in_maps = []
    for c in range(NCORES):
        sl = np.zeros((2 * E, 36), np.float32)
        for j in range(EL):
            sl[EL * c + j, j] = 1.0
            sl[E + EL * c + j, 32 + j] = 1.0
        in_maps.append({
            "x_full": x_w,
            "x_own": np.ascontiguousarray(x[TL * c:TL * (c + 1)]),
            "rwT": rwT,
            "ebias": e_bias,
            "sloc": sl,
            "wgu": np.ascontiguousarray(wgu[EL * c:EL * (c + 1)]),
            "wd": np.ascontiguousarray(wd_b[EL * c:EL * (c + 1)]),
            "sguT": sguT,
            "sdT": sdT,
        })
    return in_maps


def kernel(**inputs):
    from concourse.bass_utils import run_bass_kernel_spmd

    nc = get_nc()
    in_maps = make_in_maps(inputs)
    trace = bool(int(os.environ.get("BASS_MOE_TRACE", "0")))
    res = run_bass_kernel_spmd(
        nc, in_maps, core_ids=list(range(NCORES)), trace=trace)
    if trace and res.exec_time_ns is not None:
        print(f"HW exec time: {res.exec_time_ns} ns")
        kernel.last_exec_time_ns = res.exec_time_ns
    out = np.concatenate([res.results[c]["out"] for c in range(NCORES)], axis=0)
    return out.reshape(B, S, D)


kernel.last_exec_time_ns = None
